# revision 6
# baseline (speedup 1.0000x reference)
"""DRCLoss kernel for 8 Trainium2 NeuronCores (Bass/Tile, SPMD).

Math: loss = mean_i[ relu(l1_i + l2_i + d12_i - neg_i + 0.1) + max(l1_i, l2_i) ]
  where dist = cdist(ts, [ts; im1; im2]), l1/l2 are the block diagonals,
  neg is the min over non-self columns, d12 = ||im1 - im2 + 1e-6||.

Only `neg` needs the full [B, 3B] distance matrix; l1/l2/d12 and the row
norms are exact row-wise host work.  Device computes, per row i of its
512-row slice, max_j w_ij with w = 2*x.r_j - ||r_j||^2 (so that
min_j dist^2 = ||x_i||^2 - max_j w_ij):

  - One fp8 DoubleRow matmul pair per [128, 512] tile covers the whole
    K=512 contraction: 508 fp8 data dims (the 4 lowest-energy dims are
    dropped; abs effect on the loss ~1e-4) plus 4 fp8 "aug" rows that
    encode -||r_j||^2 to ~0.01 absolute (64a + 4b + c + e/16 split).
  - Self columns (tiles 0/8/16 after the per-core rotation, col m*128+p)
    are knocked out in PSUM by tiny fp8 eye matmuls (-448*448*2 each).
  - The masked running max runs on the only two engines that can read
    PSUM on this arch (GPSIMD cannot).  The nt loop is m-major over
    chunk pairs so PSUM holds same-m quads [128, 4, 512] (4 banks,
    bufs=2 -> all 8 banks, double buffered):
      DVE:  one tensor_reduce(max) per quad -> its own [128,1] output
            column (host folds the per-quad maxima)
      Act:  one activation(Exp, scale=beta, bias=-beta*(xsq_i-C0),
            accum_out) per quad -> sum of exp(beta*w), a log-sum-exp
            upper bound on the quad max (error <0.1 on dist^2, ~1e-5
            on the loss)
  - Host finishes in float64: neg = sqrt(xsq - max(direct, lse)), plus
    exact l1/l2/d12 and the triplet/mean.
"""

import sys

if "/opt/trn_rl_repo" not in sys.path:
    sys.path.insert(0, "/opt/trn_rl_repo")

from contextlib import ExitStack

import ml_dtypes
import numpy as np

import concourse.bass as bass
import concourse.tile as tile
from concourse import mybir
from concourse.bass_utils import run_bass_kernel_spmd

BF16 = ml_dtypes.bfloat16
F8 = ml_dtypes.float8_e4m3
F32 = np.float32

B = 4096          # rows of feature_ts
D = 512           # feature dim
M = 8             # cores
BC = B // M       # rows per core (512)
NCOL = 3 * B      # columns of the distance matrix (12288)
NCH = 12          # DMA chunks (1024 cols each)
CW = NCOL // NCH  # 1024
NT = NCOL // 512  # 24 column tiles per core
NCP = 6           # chunk pairs (4 nts each)
NDROP = 4         # data dims dropped to make room for the aug rows
BETA = 0.25       # LSE sharpness for the Act-engine quads
C0 = 850.0        # LSE centering: best-col exp arg lands in ~[-60, 40]

MARGIN = 0.1
PD_EPS = 1e-6
NEG_BIG = -3.0e38

# ---- static quad schedule ---------------------------------------------------
# Quad (cp, m) = column tiles 4cp..4cp+3 of row block m; consumer is DVE
# (exact per-quad max) or Act (exp-sum -> LSE on host).  Act must never see
# an eye-masked self column (its exp table NaNs on huge-negative args), so
# on the diag chunk pairs (cp 0/2/4, diag nt in slot 0) an "act" quad is
# split: DVE takes slot 0, Act takes the contiguous slots 1..3.
DIAG_CPS = (0, 2, 4)
DIAG_NTS = (0, 8, 16)
_ASSIGN = {}
for _cp in range(NCP):
    for _m in range(4):
        _ASSIGN[(_cp, _m)] = "act" if _m in (1, 3) else "dve"
_ASSIGN[(1, 0)] = "act"
_ASSIGN[(3, 2)] = "act"

# osq column maps (emission order: cp-major, then m)
DVE_COL_M = []                # m of each dve output column
ACT_COL_M = []                # m of each act output column
for _cp in range(NCP):
    for _m in range(4):
        if _ASSIGN[(_cp, _m)] == "dve":
            DVE_COL_M.append(_m)
        else:
            if _cp in DIAG_CPS:
                DVE_COL_M.append(_m)   # slot-0 single
            ACT_COL_M.append(_m)
NACT = len(ACT_COL_M)         # 14
NDVE = len(DVE_COL_M)         # 16 (10 quads + 6 singles)
NOUT = NDVE + NACT

LAST_RESULTS = None  # BassKernelResults of the most recent run (for test.py)

_NC_CACHE = None


def _install_ntff_hook():
    """Provide antenv.axon_hooks (missing in this image) so trace=True can
    capture NTFF profiles through libaxon_pjrt.so."""
    try:
        import antenv.axon_hooks  # noqa: F401

        return
    except ImportError:
        pass
    try:
        import types

        import antenv
        from trn_agent_boot.trn_boot import _ntff_profile_via_ctypes

        mod = types.ModuleType("antenv.axon_hooks")
        mod._hook = None

        def set_axon_ntff_profile_hook(h):
            mod._hook = h

        def get_axon_ntff_profile_hook():
            return mod._hook

        mod.set_axon_ntff_profile_hook = set_axon_ntff_profile_hook
        mod.get_axon_ntff_profile_hook = get_axon_ntff_profile_hook
        sys.modules["antenv.axon_hooks"] = mod
        antenv.axon_hooks = mod
        hook = _ntff_profile_via_ctypes("/opt/axon/libaxon_pjrt.so")
        if hook is not None:
            mod._hook = hook
    except Exception:
        pass


def _maybe_patch_ldw_opt():
    import os

    if os.environ.get("BASS_LDW_OPT") != "1":
        return
    from concourse import bass_utils as _bu

    if getattr(_bu, "_ldw_patched", False):
        return
    _orig = _bu.run_command

    def run_command(cmd, **kw):
        if isinstance(cmd, list):
            cmd = [
                "--enable-ldw-opt=true" if c == "--enable-ldw-opt=false" else c
                for c in cmd
            ]
        return _orig(cmd, **kw)

    _bu.run_command = run_command
    _bu._ldw_patched = True


def _split_multi_waits(nc):
    """This walrus build allows only ONE embedded sync wait per instruction.
    Hoist extra waits onto standalone EventSemaphore instructions inserted
    just before the owner (same engine, so program order is preserved)."""
    import bass_rust

    ctr = 0
    for blk in nc.m.functions[0].blocks:
        il = blk.instructions
        new = []
        for inst in il:
            si = getattr(inst, "sync_info", None)
            waits = list(si.on_wait) if si is not None else []
            if len(waits) > 1:
                for w in waits[:-1]:
                    ev = bass_rust.InstEventSemaphore(name=f"wsplit_{ctr}")
                    ctr += 1
                    ev.engine = inst.engine
                    ev.sync_info = bass_rust.SyncInfo(on_wait=[w], on_update=[])
                    new.append(ev)
                inst.sync_info = bass_rust.SyncInfo(
                    on_wait=[waits[-1]], on_update=list(si.on_update)
                )
            new.append(inst)
        il[:] = new
    return nc


def _build_nc():
    """Build the SPMD Bass program (identical for all cores)."""
    nc = bass.Bass()
    f32 = mybir.dt.float32
    bf16 = mybir.dt.bfloat16
    f8 = mybir.dt.float8e4
    DR = mybir.MatmulPerfMode.DoubleRow
    mx = mybir.AluOpType.max
    add = mybir.AluOpType.add

    rt8_d = nc.dram_tensor("rt8", [NCH, 128, 4, CW], f8, kind="ExternalInput")
    lt8_d = nc.dram_tensor("lt8", [128, 4, BC], f8, kind="ExternalInput")
    bias_d = nc.dram_tensor("bias", [128, 4], f32, kind="ExternalInput")
    eye8_d = nc.dram_tensor("eye8", [128, 4, 128], f8, kind="ExternalInput")
    osq_d = nc.dram_tensor("osq", [128, NOUT], f32, kind="ExternalOutput")

    with ExitStack() as ctx:
        tc = ctx.enter_context(tile.TileContext(nc))
        const = ctx.enter_context(tc.tile_pool(name="const", bufs=1))
        psump = ctx.enter_context(tc.tile_pool(name="psum", bufs=2, space="PSUM"))

        def psum_quad():
            return psump.tile([128, 2048], f32, tag="q", name="psq")

        def dummy_mm(lhs_ap, rhs_ap):
            pw = psum_quad()
            nc.tensor.matmul(pw[: lhs_ap.shape[-1], : rhs_ap.shape[-1]],
                             lhs_ap, rhs_ap, start=True, stop=True)

        # --- t=0 engine-local init (no DMA deps) --------------------------
        osq = const.tile([128, NOUT], f32, tag="osq")
        nc.vector.memset(osq, NEG_BIG)
        zr4 = const.tile([128, 8], bf16, tag="zr4")
        nc.vector.memset(zr4, 0.0)
        warm8 = const.tile([128, 512], f8, tag="warm8")
        nc.vector.memset(warm8, 1.0)

        # PE warmup on the memset tile: ~3.5us of matmul busy releases the
        # HAM p-state throttle before the first data tile.
        for _ in range(8):
            pw = psum_quad()
            nc.tensor.matmul(pw[:, 0:512], warm8[:, 0:128], warm8,
                             start=True, stop=True)

        # --- input DMAs (SP queue), with consumer-engine absorbers --------
        lt8 = const.tile([128, 4, BC], f8, tag="lt8")
        nc.sync.dma_start(out=lt8, in_=lt8_d[:, :, :])
        dummy_mm(lt8[:, 0, 0:4], lt8[:, 0, 0:8])

        rt8_t = {}

        def dma_chunk(ch):
            t8 = const.tile([128, 4, CW], f8, tag=f"rt8_{ch}", name=f"rt8_{ch}")
            nc.sync.dma_start(out=t8, in_=rt8_d[ch])
            dummy_mm(t8[:, 0, 0:4], t8[:, 0, 0:8])
            rt8_t[ch] = t8

        dma_chunk(0)
        dma_chunk(1)

        eye8 = const.tile([128, 4, 128], f8, tag="eye8")
        nc.sync.dma_start(out=eye8, in_=eye8_d[:, :, :])
        dummy_mm(eye8[:, 0, 0:4], eye8[:, 0, 0:8])
        bias_t = const.tile([128, 4], f32, tag="bias")
        nc.sync.dma_start(out=bias_t, in_=bias_d[:, :])
        # bias absorber + Exp act-table preload in one tiny instruction
        scra = const.tile([128, 8], f32, tag="scra")
        nc.scalar.activation(scra, zr4[:, 0:8],
                             mybir.ActivationFunctionType.Exp,
                             bias=bias_t[:, 0:1], scale=0.0)

        for ch in range(2, NCH):
            dma_chunk(ch)

        # --- main stream: m-major over chunk pairs ------------------------
        act_col = NDVE
        dve_col = 0
        for cp in range(NCP):
            for m in range(4):
                quad = psum_quad()
                for s in range(4):
                    nt = 4 * cp + s
                    ch, t_i = nt // 2, nt % 2
                    ps = quad[:, s * 512 : (s + 1) * 512]
                    diag = nt in DIAG_NTS
                    nc.tensor.matmul(
                        ps,
                        lt8[:, 0:2, m * 128 : (m + 1) * 128],
                        rt8_t[ch][:, 0:2, t_i * 512 : (t_i + 1) * 512],
                        start=True,
                        stop=False,
                        perf_mode=DR,
                    )
                    if diag:
                        # knock the self column (m*128+p) out of the max
                        nc.tensor.matmul(
                            quad[:, s * 512 + m * 128 : s * 512 + (m + 1) * 128],
                            eye8[:, 0:2, :],
                            eye8[:, 2:4, :],
                            start=False,
                            stop=False,
                            perf_mode=DR,
                        )
                    nc.tensor.matmul(
                        ps,
                        lt8[:, 2:4, m * 128 : (m + 1) * 128],
                        rt8_t[ch][:, 2:4, t_i * 512 : (t_i + 1) * 512],
                        start=False,
                        stop=True,
                        perf_mode=DR,
                    )
                if _ASSIGN[(cp, m)] == "dve":
                    nc.vector.tensor_reduce(
                        out=osq[:, dve_col : dve_col + 1],
                        in_=quad,
                        axis=mybir.AxisListType.X,
                        op=mx,
                    )
                    dve_col += 1
                else:
                    lo = 0
                    if cp in DIAG_CPS:
                        # DVE takes the diag slot; Act must not exp the mask
                        nc.vector.tensor_reduce(
                            out=osq[:, dve_col : dve_col + 1],
                            in_=quad[:, 0:512],
                            axis=mybir.AxisListType.X,
                            op=mx,
                        )
                        dve_col += 1
                        lo = 512
                    nc.scalar.activation(
                        quad[:, lo:2048],
                        quad[:, lo:2048],
                        mybir.ActivationFunctionType.Exp,
                        bias=bias_t[:, m : m + 1],
                        scale=BETA,
                        accum_out=osq[:, act_col : act_col + 1],
                    )
                    act_col += 1

        # SWDGE: a fresh queue, so the only wait is the data dep
        nc.gpsimd.dma_start(out=osq_d[:, :], in_=osq)

    _split_multi_waits(nc)
    return nc


def _prep(feature_ts, feature_image1, feature_image2):
    """Host: fp8 casts, aug rows, rotation, plus exact row-wise terms."""
    ts = np.ascontiguousarray(feature_ts, dtype=np.float32)
    im1 = np.ascontiguousarray(feature_image1, dtype=np.float32)
    im2 = np.ascontiguousarray(feature_image2, dtype=np.float32)

    R = np.concatenate([ts, im1, im2], 0)                   # [3B, D]
    rsq = (R.astype(np.float64) ** 2).sum(1)
    xsq = (ts.astype(np.float64) ** 2).sum(1)

    x8 = ts.astype(F8)                                      # [B, D]
    r8f = R.astype(F8).astype(np.float32)
    r28 = (2.0 * r8f).astype(F8)                            # exact doubling

    # drop the NDROP lowest-energy dims to make room for the aug rows
    energy = (x8.astype(np.float32) ** 2).sum(0) + (r8f**2).sum(0)
    keep = np.sort(np.argsort(energy)[NDROP:])

    # -rsq ~ -(64a + 4b + c + e/16); a,b integer-exact in fp8
    a = np.round(rsq / 64.0)
    b = np.round((rsq - 64 * a) / 4.0)
    c = (rsq - 64 * a - 4 * b).astype(F8)
    e = ((rsq - 64 * a - 4 * b - c.astype(np.float64)) * 16.0).astype(F8)

    right = np.empty((512, NCOL), dtype=F8)                 # [slot, col]
    right[0 : 512 - NDROP] = r28[:, keep].T
    right[508] = a.astype(F8)
    right[509] = b.astype(F8)
    right[510] = c
    right[511] = e
    rt8_full = np.ascontiguousarray(
        right.reshape(4, 128, NCOL).transpose(1, 0, 2)      # [p, j, col]
    )

    left = np.empty((512, B), dtype=F8)                     # [slot, row]
    left[0 : 512 - NDROP] = x8[:, keep].T
    left[508] = F8(-64.0)
    left[509] = F8(-4.0)
    left[510] = F8(-1.0)
    left[511] = F8(-0.0625)
    lt8_full = np.ascontiguousarray(left.reshape(4, 128, B).transpose(1, 0, 2))

    bias_full = (-BETA * (xsq.astype(np.float32) - np.float32(C0))).astype(
        np.float32
    )                                                       # [B]

    # eye8: lhsT rows (j 0:2) = -448*I, rhs rows (j 2:4) = +448*I
    eye8 = np.zeros((128, 4, 128), dtype=F8)
    p = np.arange(128)
    for j in range(2):
        eye8[p, j, p] = F8(-448.0)
        eye8[p, 2 + j, p] = F8(448.0)

    base = np.arange(B)
    in_maps = []
    for cidx in range(M):
        perm = np.concatenate([blk * B + (base + cidx * BC) % B for blk in range(3)])
        rt8_c = rt8_full[:, :, perm]                        # [128, 4, NCOL]
        rt8_c = np.ascontiguousarray(
            rt8_c.reshape(128, 4, NCH, CW).transpose(2, 0, 1, 3)
        )                                                   # [NCH, 128, 4, CW]
        in_maps.append(
            {
                "rt8": rt8_c,
                "lt8": np.ascontiguousarray(
                    lt8_full[:, :, cidx * BC : (cidx + 1) * BC]
                ),
                "bias": np.ascontiguousarray(
                    bias_full.reshape(M, 4, 128)[cidx].T
                ),
                "eye8": eye8,
            }
        )

    # exact row-wise pieces for the host epilogue
    l1 = np.sqrt(((ts.astype(np.float64) - im1) ** 2).sum(1))
    l2 = np.sqrt(((ts.astype(np.float64) - im2) ** 2).sum(1))
    d12 = np.sqrt(((im1.astype(np.float64) - im2 + PD_EPS) ** 2).sum(1))
    host = {"xsq": xsq, "l1": l1, "l2": l2, "d12": d12, "bias": bias_full}
    return in_maps, host


def _combine(osq_list, host):
    """Host epilogue in float64: neg from the device maxima, then the loss."""
    xsq, l1, l2, d12 = host["xsq"], host["l1"], host["l2"], host["d12"]
    bias = host["bias"].astype(np.float64)
    act_m = np.asarray(ACT_COL_M)                           # [NACT]
    dve_m = np.asarray(DVE_COL_M)                           # [NDVE]
    maxw = np.empty(B)
    for cidx in range(M):
        o = np.asarray(osq_list[cidx], dtype=np.float64)    # [128, NOUT]
        for m in range(4):
            rows = cidx * BC + m * 128 + np.arange(128)
            dcols = np.nonzero(dve_m == m)[0]
            direct = (
                o[:, dcols].max(1) if dcols.size else np.full(128, -np.inf)
            )
            cols = NDVE + np.nonzero(act_m == m)[0]
            s = o[:, cols].sum(1)
            lse = np.where(
                s > 0.0,
                (np.log(np.maximum(s, 1e-300)) - bias[rows]) / BETA,
                -np.inf,
            )
            maxw[rows] = np.maximum(direct, lse)
    neg = np.sqrt(np.maximum(xsq - maxw, 0.0))
    trip = np.maximum(l1 + l2 + d12 - neg + MARGIN, 0.0) + np.maximum(l1, l2)
    return np.float32(trip.sum() / B)


def kernel(feature_ts, feature_image1, feature_image2, _trace=False):
    global _NC_CACHE, LAST_RESULTS
    if _NC_CACHE is None:
        _NC_CACHE = _build_nc()
    if _trace:
        _install_ntff_hook()
    _maybe_patch_ldw_opt()
    in_maps, host = _prep(feature_ts, feature_image1, feature_image2)
    res = run_bass_kernel_spmd(_NC_CACHE, in_maps, list(range(M)), trace=_trace)
    LAST_RESULTS = res
    return _combine([res.results[c]["osq"] for c in range(M)], host)
